# revision 3
# baseline (speedup 1.0000x reference)
"""CAREConv GNN layer on 8 TRN2 NeuronCores — v2.

Algorithm (per edge type, 3 types):
  mlp_h = tanh(feat @ W_mlp + b_mlp)            [N, 2]
  d[n,k] = ||mlp_h[nbr[n,k]] - mlp_h[n]||_1     [N, 32]
  keep 16 smallest-d neighbors (ties -> smaller k), agg = tanh(mean(feat[sel]))
  out = (0.5*(agg0+agg1+agg2) + feat) -> tanh -> @ W_lin + b_lin

v2 design (vs v1):
  - No collective: every core computes mlp_h for ALL N nodes from a
    pre-transposed f32 feat table (98 small matmuls) — removes the
    AllGather and all cross-core dependencies.
  - u-phase distances via on-chip gpsimd ap_gather from an SBUF-resident
    fp16 table [128, 25000 pairs, 2] (partition p holds channel p%2),
    instead of 600k HBM dma_gather descriptors per core.
  - Distances are compared as dpv = 1 - d/4 - k*2^-22 (exact tie-break by
    k, matching jax.lax.top_k stable semantics on quantized-consistent d).
  - feat aggregation keeps the proven pair-table dma_gather + parity-mask
    path in dst-node layout.
  - Tiles processed in interleaved groups (distances for a group under the
    ap_gather library, then select+aggregate for that group under the mlp
    library) so the Pool-bound u-phase of group g+1 overlaps the SDMA-bound
    feat gathers of group g.
"""

import os
import numpy as np
import ml_dtypes

import concourse.bass as bass
import concourse.mybir as mybir
import concourse.tile as tile
import concourse.bacc as bacc
from concourse.library_config import mlp as mlp_lib
from concourse.library_config import ap_gather as apg_lib
from concourse.bass_utils import run_bass_kernel_spmd
from concourse.masks import make_identity

F32 = mybir.dt.float32
FP16 = mybir.dt.float16
BF16 = mybir.dt.bfloat16
I16 = mybir.dt.int16
I32 = mybir.dt.int32

N = 50000
D = 128
K = 32
SEL = 16
CORES = 8
NS = N // CORES            # 6250 dst nodes per core
T = 49                     # tiles of 128 (6272 padded)
NT = T * 128
DO = 64
W = N // 2                 # 25000 pair entries
E3 = 3
EK = E3 * K                # 96 edges per node
NIU = 16 * EK + 16         # ap_gather idxs per core per tile: 1536 edges + 16 dst
NIF = E3 * SEL * 128       # feat-gather idxs per tile = 6144


def split_excess_waits(nc, max_waits=1):
    for fn in nc.m.functions:
        for bb in list(fn.blocks):
            out = []
            for ins in bb.instructions:
                si = ins.sync_info
                if si is not None and si.on_wait and len(si.on_wait) > max_waits:
                    waits = list(si.on_wait)
                    extra = waits[:-max_waits]
                    for i in range(0, len(extra), max_waits):
                        nop = nc.engines[ins.engine].nop(nofuse=True).ins
                        popped = False
                        for b2 in fn.blocks:
                            if b2.instructions and b2.instructions[-1] is nop:
                                b2.instructions.pop()
                                popped = True
                                break
                        assert popped
                        nop.sync_info = mybir.SyncInfo(
                            on_wait=extra[i : i + max_waits], on_update=[]
                        )
                        out.append(nop)
                    si.on_wait = waits[-max_waits:]
                out.append(ins)
            bb.instructions[:] = out


def ap_(t_ap, off, dims):
    """Manual AP: keep the tile's partition dim, set free dims [[step, count],...]."""
    return bass.AP(t_ap.tensor, t_ap.offset + off, [list(t_ap.ap[0])] + [list(d) for d in dims])


def build_nc(repeat=1, single_packet=False, grp=7):
    nc = bacc.Bacc("TRN2", target_bir_lowering=False, debug=False, num_devices=CORES, num_swdge_queues=4)

    feat_pair = nc.dram_tensor("feat_pair", [W, 256], BF16, kind="ExternalInput")
    feat_dst = nc.dram_tensor("feat_dst", [NT, D], F32, kind="ExternalInput")
    featT_in = nc.dram_tensor("featT", [D, N], F32, kind="ExternalInput")
    wmlp_in = nc.dram_tensor("wmlp", [D, 2], F32, kind="ExternalInput")
    bmlpT_in = nc.dram_tensor("bmlpT", [D, 1], F32, kind="ExternalInput")
    wlin_in = nc.dram_tensor("wlin", [D, DO], F32, kind="ExternalInput")
    blin_in = nc.dram_tensor("blin", [128, DO], F32, kind="ExternalInput")
    idxg_in = nc.dram_tensor("idxg", [T, 128, NIU // 16], I16, kind="ExternalInput")
    kb_in = nc.dram_tensor("keybase", [T, 128, EK], F32, kind="ExternalInput")
    selq_in = nc.dram_tensor("selq", [8, 128, 128], F32, kind="ExternalInput")
    ck_in = nc.dram_tensor("ck", [128, EK], F32, kind="ExternalInput")
    out_d = nc.dram_tensor("out", [NT, DO], F32, kind="ExternalOutput")

    AF = mybir.ActivationFunctionType
    OP = mybir.AluOpType

    with tile.TileContext(nc) as tc:
        with (
            tc.tile_pool(name="const", bufs=1) as cp,
            tc.tile_pool(name="resid", bufs=1) as rp,
            tc.tile_pool(name="sa", bufs=2) as sa,
            tc.tile_pool(name="biga", bufs=2) as biga,
            tc.tile_pool(name="sb", bufs=2) as sb,
            tc.tile_pool(name="bigb", bufs=2) as bigb,
            tc.tile_pool(name="ps", bufs=2, space="PSUM") as ps,
            tc.tile_pool(name="psq", bufs=2, space="PSUM") as psq,
        ):
            ident = cp.tile([128, 128], F32)
            make_identity(nc, ident[:])
            wmlp_sb = cp.tile([128, 2], F32)
            nc.sync.dma_start(wmlp_sb[:], wmlp_in[:])
            bmlpT_sb = cp.tile([128, 1], F32)
            nc.sync.dma_start(bmlpT_sb[:], bmlpT_in[:])
            wlin_sb = cp.tile([128, DO], F32)
            nc.sync.dma_start(wlin_sb[:], wlin_in[:])
            blin_sb = cp.tile([128, DO], F32)
            nc.sync.dma_start(blin_sb[:], blin_in[:])
            selq_sb = cp.tile([128, 8 * 128], F32)
            nc.sync.dma_start(
                selq_sb[:].rearrange("p (q c) -> p q c", q=8),
                selq_in[:].rearrange("q p c -> p q c"),
            )
            ck_sb = cp.tile([128, EK], F32)
            nc.sync.dma_start(ck_sb[:], ck_in[:])

            # dd_all[dst_local_p, t*EK + (e,k)] = |d0|+|d1| (fp16-consistent, f32 sums)
            dd_all = rp.tile([128, T * EK], F32)
            # utab[p, w, j] = fp16(mlp_h[2w + j, p % 2]) — SBUF-resident all kernel
            utab = rp.tile([128, 2 * W], FP16)

            nc.gpsimd.load_library(apg_lib)

            # ================= phase 0: mlp table =================
            with (
                tc.tile_pool(name="p0", bufs=2) as p0,
                tc.tile_pool(name="ps0", bufs=2, space="PSUM") as ps0,
            ):
                CB = 512
                nblk = (N + CB - 1) // CB  # 98
                for b in range(nblk):
                    cols = min(CB, N - b * CB)
                    ft = p0.tile([128, CB], F32, tag="ft")
                    nc.sync.dma_start(ft[:, :cols], featT_in[:, b * CB : b * CB + cols])
                    mh_ps = ps0.tile([2, CB], F32, tag="mh")
                    nc.tensor.matmul(mh_ps[:, :cols], lhsT=wmlp_sb[:], rhs=ft[:, :cols], start=True, stop=True)
                    nc.scalar.activation(
                        ap_(utab[0:2], b * CB, [[1, cols]]),
                        mh_ps[:, :cols],
                        AF.Tanh,
                        bias=bmlpT_sb[0:2],
                    )
                s = 2
                while s < 128:
                    nc.sync.dma_start(utab[s : 2 * s, :], utab[0:s, :])
                    s *= 2

            utab3 = utab[:].rearrange("p (w j) -> p w j", j=2)
            qrr = [0]

            def phase_a(t):
                idx_t = sa.tile([128, NIU // 16], I16, tag="idx")
                nc.sync.dma_start(idx_t[:], idxg_in[t])
                kb_t = sa.tile([128, EK], F32, tag="kb")
                nc.sync.dma_start(kb_t[:], kb_in[t])

                G = biga.tile([128, NIU * 2], FP16, tag="G")
                nc.gpsimd.ap_gather(
                    G[:].rearrange("p (i j) -> p i j", j=2),
                    utab3,
                    idx_t[:],
                    channels=128,
                    num_elems=W,
                    d=2,
                    num_idxs=NIU,
                )
                A = biga.tile([128, 16 * EK * 2], F32, tag="A")
                for up in range(2):  # u parity: dst parity = u & 1
                    nc.vector.tensor_tensor(
                        out=ap_(A[:], up * 2 * EK, [[4 * EK, 8], [1, 2 * EK]]),
                        in0=ap_(G[:], up * 2 * EK, [[4 * EK, 8], [1, 2 * EK]]),
                        in1=ap_(G[:], 2 * 16 * EK + up * 2 + up, [[4, 8], [0, 2 * EK]]),
                        op=OP.subtract,
                    )
                nc.scalar.activation(A[:], A[:], AF.Abs)
                D2 = sa.tile([128, 2 * 2 * EK], F32, tag="D2")
                for ch in range(2):
                    nc.sync.dma_start(
                        ap_(D2[:], ch * 2 * EK, [[1, 2 * EK]]),
                        ap_(A[ch:128:16], 0, [[2 * EK, 16], [1, 2 * EK]]),
                    )
                ddb = sa.tile([128, 2 * EK], F32, tag="ddb")
                nc.vector.tensor_tensor(
                    out=ddb[:], in0=D2[:, 0 : 2 * EK], in1=D2[:, 2 * EK : 4 * EK], op=OP.add
                )
                kbi = sa.tile([128, EK], I32, tag="kbi")
                nc.vector.tensor_copy(kbi[:], kb_t[:])
                kpar = sa.tile([128, EK], I32, tag="kpar")
                nc.vector.tensor_scalar(kpar[:], kbi[:], 1, None, OP.bitwise_and)
                ddt = ap_(dd_all[:], t * EK, [[1, EK]])
                nc.vector.tensor_copy(ddt, ap_(ddb[:], 1, [[2, EK]]))  # odd v
                nc.vector.copy_predicated(ddt, kpar[:], ap_(ddb[:], 0, [[2, EK]]))

            def phase_b(t):
                kb_t = sb.tile([128, EK], F32, tag="kb2")
                nc.sync.dma_start(kb_t[:], kb_in[t])
                fd = sb.tile([128, D], F32, tag="fd")
                nc.sync.dma_start(fd[:], feat_dst[t * 128 : (t + 1) * 128, :])

                dpv = sb.tile([128, EK], F32, tag="dpv")
                nc.vector.tensor_scalar(
                    dpv[:], ap_(dd_all[:], t * EK, [[1, EK]]), -0.25, 1.0, OP.mult, OP.add
                )
                nc.vector.tensor_tensor(out=dpv[:], in0=dpv[:], in1=ck_sb[:], op=OP.add)

                w1 = sb.tile([128, EK], F32, tag="w1")
                w2 = sb.tile([128, EK], F32, tag="w2")
                for e in range(E3):
                    dpe = ap_(dpv[:], K * e, [[1, K]])
                    w1e = ap_(w1[:], K * e, [[1, K]])
                    w2e = ap_(w2[:], K * e, [[1, K]])
                    mx = sb.tile([128, 8], F32, tag="mx")
                    nc.vector.max(mx[:], dpe)
                    nc.vector.match_replace(w1e, in_to_replace=mx[:], in_values=dpe, imm_value=0.0)
                    mx2 = sb.tile([128, 8], F32, tag="mx2")
                    nc.vector.max(mx2[:], w1e)
                    nc.vector.match_replace(w2e, in_to_replace=mx2[:], in_values=w1e, imm_value=0.0)
                mask = sb.tile([128, EK], F32, tag="mask")
                nc.vector.tensor_scalar(mask[:], w2[:], 0.0, None, OP.is_equal)
                key = sb.tile([128, EK], F32, tag="key")
                nc.vector.tensor_tensor(out=key[:], in0=mask[:], in1=kb_t[:], op=OP.mult)
                sk = sb.tile([128, E3 * SEL], F32, tag="sk")
                kz = sb.tile([128, EK], F32, tag="kz")
                for e in range(E3):
                    keye = ap_(key[:], K * e, [[1, K]])
                    kze = ap_(kz[:], K * e, [[1, K]])
                    sk1 = ap_(sk[:], SEL * e, [[1, 8]])
                    sk2 = ap_(sk[:], SEL * e + 8, [[1, 8]])
                    mxa = sb.tile([128, 8], F32, tag="mxa")
                    nc.vector.max(mxa[:], keye)
                    nc.vector.tensor_copy(sk1, mxa[:])
                    nc.vector.match_replace(kze, in_to_replace=mxa[:], in_values=keye, imm_value=0.0)
                    mxb = sb.tile([128, 8], F32, tag="mxb")
                    nc.vector.max(mxb[:], kze)
                    nc.vector.tensor_copy(sk2, mxb[:])
                vt = sb.tile([128, E3 * SEL], F32, tag="vt")
                nc.vector.tensor_scalar(vt[:], sk[:], 1.0, None, OP.subtract)
                vti = sb.tile([128, E3 * SEL], I32, tag="vti")
                nc.vector.tensor_copy(vti[:], vt[:])
                vi = sb.tile([128, E3 * SEL], I32, tag="vi")
                nc.vector.tensor_scalar(vi[:], vti[:], 65535, None, OP.bitwise_and)
                vsel = sb.tile([128, E3 * SEL], F32, tag="vsel")
                nc.vector.tensor_copy(vsel[:], vi[:])
                pi = sb.tile([128, E3 * SEL], I32, tag="pi")
                nc.vector.tensor_scalar(pi[:], vi[:], 1, None, OP.bitwise_and)
                psel = sb.tile([128, E3 * SEL], F32, tag="psel")
                nc.vector.tensor_copy(psel[:], pi[:])
                wsel = sb.tile([128, E3 * SEL], F32, tag="wsel")
                nc.vector.tensor_tensor(out=wsel[:], in0=vsel[:], in1=psel[:], op=OP.subtract)
                nc.vector.tensor_scalar(wsel[:], wsel[:], 0.5, None, OP.mult)
                widx = sb.tile([128, NIF // 16], I16, tag="widx")
                for q in range(8):
                    pq = psq.tile([128, E3 * SEL], F32, tag="pq")
                    nc.tensor.matmul(
                        pq[:],
                        lhsT=ap_(selq_sb[:], q * 128, [[1, 128]]),
                        rhs=wsel[:],
                        start=True,
                        stop=True,
                    )
                    nc.vector.tensor_copy(ap_(widx[:], q, [[8, E3 * SEL]]), pq[:])
                # parity weights
                wm = sb.tile([128, E3 * SEL * 2], F32, tag="wm")
                nc.vector.tensor_copy(ap_(wm[:], 1, [[2, E3 * SEL]]), psel[:])
                nc.vector.tensor_scalar(
                    ap_(wm[:], 0, [[2, E3 * SEL]]), psel[:], -1.0, 1.0,
                    OP.mult, OP.add,
                )
                agg = sb.tile([128, E3 * D], F32, tag="agg")
                for e in range(E3):
                    gfe = bigb.tile([128, SEL * 256], BF16, tag="gf")
                    nc.gpsimd.dma_gather(
                        ap_(gfe[:], 0, [[256, SEL], [1, 256]]),
                        feat_pair[:],
                        ap_(widx[:], 128 * e, [[1, 128]]),
                        NIF // E3,
                        NIF // E3,
                        256,
                        single_packet=single_packet,
                        queue_num=qrr[0] % 4,
                    )
                    qrr[0] += 1
                    nc.vector.tensor_tensor(
                        out=ap_(gfe[:], 0, [[256, SEL], [128, 2], [1, 128]]),
                        in0=ap_(gfe[:], 0, [[256, SEL], [128, 2], [1, 128]]),
                        in1=ap_(wm[:], 2 * SEL * e, [[2, SEL], [1, 2], [0, 128]]),
                        op=OP.mult,
                    )
                    nc.vector.tensor_reduce(
                        ap_(agg[:], e * D, [[D, 1], [1, D]]),
                        ap_(gfe[:], 0, [[SEL * 256, 1], [1, 128], [256, SEL], [128, 2]]),
                        axis=mybir.AxisListType.XY,
                        op=OP.add,
                    )
                nc.scalar.activation(agg[:], agg[:], AF.Tanh, scale=1.0 / SEL)
                s3 = sb.tile([128, D], F32, tag="s3")
                nc.vector.tensor_reduce(
                    s3[:].rearrange("p (f o) -> p f o", o=1),
                    ap_(agg[:], 0, [[1, D], [D, E3]]),
                    axis=mybir.AxisListType.X,
                    op=OP.add,
                )
                h1 = sb.tile([128, D], F32, tag="h1")
                nc.vector.tensor_scalar(h1[:], s3[:], 0.5, None, OP.mult)
                nc.vector.tensor_tensor(out=h1[:], in0=h1[:], in1=fd[:], op=OP.add)
                h2 = sb.tile([128, D], F32, tag="h2")
                nc.scalar.activation(h2[:], h1[:], AF.Tanh)
                psH = ps.tile([128, 128], F32, tag="ptr")
                nc.tensor.transpose(psH[:], h2[:], ident[:])
                h2T = sb.tile([128, 128], F32, tag="h2T")
                nc.vector.tensor_copy(h2T[:], psH[:])
                og = psq.tile([128, DO], F32, tag="og")
                nc.tensor.matmul(og[:], lhsT=h2T[:], rhs=wlin_sb[:], start=True, stop=True)
                osb = sb.tile([128, DO], F32, tag="osb")
                nc.vector.tensor_tensor(out=osb[:], in0=og[:], in1=blin_sb[:], op=OP.add)
                nc.sync.dma_start(out_d[t * 128 : (t + 1) * 128, :], osb[:])

            tiles = [tt for _ in range(repeat) for tt in range(T)]
            first = True
            for g0 in range(0, len(tiles), grp):
                group = tiles[g0 : g0 + grp]
                if not first:
                    nc.gpsimd.load_library(apg_lib)
                first = False
                for t in group:
                    phase_a(t)
                nc.gpsimd.load_library(mlp_lib)
                for t in group:
                    phase_b(t)

    nc.compile()
    split_excess_waits(nc)
    return nc


_CACHE = {}


def _host_prep(feat, W_mlp, b_mlp, W_lin, b_lin, nbr0, nbr1, nbr2):
    feat = np.asarray(feat, dtype=np.float32)
    nbrs = np.stack([np.asarray(x, dtype=np.int32) for x in (nbr0, nbr1, nbr2)])  # [3, N, K]
    feat_pair = np.ascontiguousarray(
        feat.astype(ml_dtypes.bfloat16).reshape(W, 256)
    )
    featT = np.ascontiguousarray(feat.T)  # [128, N] f32
    selq = np.zeros((8, 128, 128), np.float32)
    p2 = np.arange(128)
    for q in range(8):
        selq[q, q * 16 + (p2 % 16), p2] = 1.0

    wmlp = np.ascontiguousarray(np.asarray(W_mlp, np.float32))
    bmlpT = np.zeros((128, 1), np.float32)
    bmlpT[0:2, 0] = np.asarray(b_mlp, np.float32)
    wlin = np.ascontiguousarray(np.asarray(W_lin, np.float32))
    blin = np.broadcast_to(np.asarray(b_lin, np.float32), (128, DO)).copy()
    kk = np.arange(K, dtype=np.float32)
    ck = np.broadcast_to(
        (-(kk * (2.0 ** -22)))[None, None, :], (128, E3, K)
    ).reshape(128, EK).copy()

    in_maps = []
    for c in range(CORES):
        sl = slice(c * NS, (c + 1) * NS)
        fd = np.zeros((NT, D), np.float32)
        fd[:NS] = feat[sl]
        nb = np.zeros((E3, NT, K), np.int32)
        for e in range(E3):
            nb[e, :NS] = nbrs[e][sl]
        # u-phase ap_gather idx: per tile t, group g, i in [0,1552)
        #   i<1536: (u=i//96, j=i%96 -> e=j//32, k=j%32), dst=16g+u -> src pair
        #   i>=1536: u=i-1536 -> dst node pair
        # wrapped: idxg[t, 16g + i%16, i//16]
        src = nb.transpose(1, 0, 2).reshape(NT, EK)          # [NT, 96] src per (dst, e*K+k)
        srcp = (src >> 1).astype(np.int16)                   # pair idx
        srcp = srcp.reshape(T, 8, 16, EK)                    # [t, g, u, j]
        edge_part = srcp.transpose(0, 1, 3, 2).reshape(T, 8, 96, 16)  # i//16 = u-major? no:
        # i = u*96 + j -> i%16 = (u*96+j)%16 = (j + 2u... careful: build explicitly
        iu = np.arange(16 * EK)
        u_of_i = iu // EK
        j_of_i = iu % EK
        ew = srcp[:, :, u_of_i, j_of_i]                      # [T, 8, 1536] in i order
        dst_ids = np.minimum(c * NS + np.arange(NT), N - 1).astype(np.int32)
        dstp = (dst_ids >> 1).astype(np.int16).reshape(T, 8, 16)  # [t, g, u]
        allidx = np.concatenate([ew, dstp], axis=2)          # [T, 8, 1552] in i order
        idxg = np.zeros((T, 8, 16, NIU // 16), np.int16)
        ii = np.arange(NIU)
        idxg[:, :, ii % 16, ii // 16] = allidx
        idxg = idxg.reshape(T, 128, NIU // 16)
        # keybase [T, 128, 96]
        kkk = np.arange(K)[None, None, :]
        kb = ((31 - kkk) * 65536 + nb + 1).astype(np.float32)  # [e, NT, K]
        kb = kb.reshape(E3, T, 128, K).transpose(1, 2, 0, 3).reshape(T, 128, EK)
        in_maps.append(
            {
                "feat_pair": feat_pair,
                "feat_dst": fd,
                "featT": featT,
                "wmlp": wmlp,
                "bmlpT": bmlpT,
                "wlin": wlin,
                "blin": blin,
                "idxg": np.ascontiguousarray(idxg),
                "keybase": np.ascontiguousarray(kb),
                "selq": selq,
                "ck": ck,
            }
        )
    return in_maps


def bench(feat, W_mlp, b_mlp, W_lin, b_lin, nbr0, nbr1, nbr2, iters=5, repeat=1):
    """Timed repeated execution with device-resident inputs. Returns (ns, out, times)."""
    import time
    import jax
    from jax.sharding import Mesh, PartitionSpec, NamedSharding
    from jax.experimental.shard_map import shard_map
    from concourse import bass2jax

    key = ("nc", repeat)
    if key not in _CACHE:
        _CACHE[key] = build_nc(repeat=repeat)
    nc = _CACHE[key]
    in_maps = _host_prep(feat, W_mlp, b_mlp, W_lin, b_lin, nbr0, nbr1, nbr2)

    bass2jax.install_neuronx_cc_hook()
    partition_name = nc.partition_id_tensor.name if nc.partition_id_tensor else None
    import concourse.mybir as mybir_
    in_names, out_names, out_avals, zero_outs = [], [], [], []
    for alloc in nc.m.functions[0].allocations:
        if not isinstance(alloc, mybir_.MemoryLocationSet):
            continue
        name = alloc.memorylocations[0].name
        if alloc.kind == "ExternalInput":
            if name != partition_name:
                in_names.append(name)
        elif alloc.kind == "ExternalOutput":
            shape = tuple(alloc.tensor_shape)
            dtype = mybir_.dt.np(alloc.dtype)
            out_names.append(name)
            out_avals.append(jax.core.ShapedArray(shape, dtype))
            zero_outs.append(np.zeros(shape, dtype))
    n_params = len(in_names)
    n_outs = len(out_avals)
    all_in_names = list(in_names) + list(out_names)
    if partition_name is not None:
        all_in_names.append(partition_name)

    def _body(*args):
        operands = list(args)
        if partition_name is not None:
            operands.append(bass2jax.partition_id_tensor())
        outs = bass2jax._bass_exec_p.bind(
            *operands,
            out_avals=tuple(out_avals),
            in_names=tuple(all_in_names),
            out_names=tuple(out_names),
            lowering_input_output_aliases=(),
            sim_require_finite=True,
            sim_require_nnan=True,
            nc=nc,
        )
        return tuple(outs)

    devices = jax.devices()[:CORES]
    mesh = Mesh(np.asarray(devices), ("core",))
    spec = PartitionSpec("core")
    sharded = jax.jit(
        shard_map(_body, mesh=mesh, in_specs=(spec,) * (n_params + n_outs),
                  out_specs=(spec,) * n_outs, check_rep=False),
        keep_unused=True,
    )
    sh = NamedSharding(mesh, spec)
    dev_in = [
        jax.device_put(
            np.concatenate([np.asarray(in_maps[c][nm]) for c in range(CORES)], axis=0), sh
        )
        for nm in in_names
    ]
    dev_zeros = [
        jax.device_put(np.zeros((CORES * z.shape[0], *z.shape[1:]), z.dtype), sh)
        for z in zero_outs
    ]
    out = sharded(*dev_in, *dev_zeros)
    jax.block_until_ready(out)
    times = []
    for _ in range(iters):
        t0 = time.perf_counter()
        out = sharded(*dev_in, *dev_zeros)
        jax.block_until_ready(out)
        times.append(time.perf_counter() - t0)
    ns = int(min(times) * 1e9)
    res = np.asarray(out[out_names.index("out")]).reshape(CORES, NT, DO)
    full = np.concatenate([res[c][:NS] for c in range(CORES)], axis=0).astype(np.float32)
    return ns, full, times


def kernel(feat, W_mlp, b_mlp, W_lin, b_lin, nbr0, nbr1, nbr2):
    if "nc" not in _CACHE:
        _CACHE["nc"] = build_nc()
    nc = _CACHE["nc"]
    in_maps = _host_prep(feat, W_mlp, b_mlp, W_lin, b_lin, nbr0, nbr1, nbr2)
    trace = bool(os.environ.get("BASS_KERNEL_PROFILE"))
    res = run_bass_kernel_spmd(nc, in_maps, list(range(CORES)), trace=trace)
    if trace:
        _CACHE["last_exec_ns"] = res.exec_time_ns
    out = np.concatenate([res.results[c]["out"][:NS] for c in range(CORES)], axis=0)
    return out.astype(np.float32)
